# revision 1
# baseline (speedup 1.0000x reference)
"""KAN layer kernel for TRN2, 8-core SPMD.

Math: out[b,o] = sum_{i,k} relu(x[b,i]*w1[o,i,k] + b1[o,i,k]) * w2[o,i,k] / 32 + b2[o]
With b1 == 0 (guaranteed by the generator) the relu factorizes:
    relu(x*w) = max(x,0)*max(w,0) + min(x,0)*min(w,0)
so the whole layer collapses to two matmuls with preprocessed weights:
    Ap[o,i] = sum_k max(w1,0)*w2      Am[o,i] = sum_k min(w1,0)*w2
    out = (max(x,0) @ Ap^T + min(x,0) @ Am^T) / 32 + b2

Sharding: 4 batch groups x 2 dout groups (core = bi*2 + oj).
Per core: x^T shard [256, 512], w1/w2 slabs [256, 4, 128] (din, k, dout-slab),
all weight preprocessing done on-device; output is out^T [128, 512].
"""

import numpy as np

B, DIN, DOUT, K = 2048, 256, 256, 4
N_CORES = 8
BG, OG = 4, 2                      # batch groups x dout groups
BS, OS = B // BG, DOUT // OG       # 512 batch rows, 128 dout cols per core
SCALE = 1.0 / np.sqrt(((DOUT + DIN) / 2) * K)   # 1/32

_CACHE = {}


def _build_nc():
    if "nc" in _CACHE:
        return _CACHE["nc"]
    import concourse.bacc as bacc
    import concourse.tile as tile
    from concourse import mybir

    f32 = mybir.dt.float32
    nc = bacc.Bacc("TRN2", target_bir_lowering=False, debug=False,
                   num_devices=N_CORES)
    xt = nc.dram_tensor("xt", [DIN, BS], f32, kind="ExternalInput")
    w1t = nc.dram_tensor("w1t", [DIN, K, OS], f32, kind="ExternalInput")
    w2t = nc.dram_tensor("w2t", [DIN, K, OS], f32, kind="ExternalInput")
    b2s = nc.dram_tensor("b2s", [OS, 1], f32, kind="ExternalInput")
    outt = nc.dram_tensor("outt", [OS, BS], f32, kind="ExternalOutput")

    AF = mybir.ActivationFunctionType
    OP = mybir.AluOpType
    NT = DIN // 128                 # i-tiles

    with tile.TileContext(nc) as tc:
        with (
            tc.tile_pool(name="io", bufs=1) as io,
            tc.tile_pool(name="work", bufs=1) as work,
            tc.tile_pool(name="pp", bufs=1, space="PSUM") as pp,
        ):
            x_t, w1_t, w2_t = [], [], []
            HB = BS // 2
            for t in range(NT):
                w1i = io.tile([128, K, OS], f32, tag=f"w1{t}")
                nc.sync.dma_start(out=w1i, in_=w1t[t * 128:(t + 1) * 128, :, :])
                w1_t.append(w1i)
                w2i = io.tile([128, K, OS], f32, tag=f"w2{t}")
                nc.sync.dma_start(out=w2i, in_=w2t[t * 128:(t + 1) * 128, :, :])
                w2_t.append(w2i)
            # x halves: [i-tile][half], ordered so half 0 lands first
            xh = [[None, None] for _ in range(NT)]
            for h in range(2):
                for t in range(NT):
                    xi = io.tile([128, HB], f32, tag=f"x{t}{h}")
                    nc.sync.dma_start(
                        out=xi,
                        in_=xt[t * 128:(t + 1) * 128, h * HB:(h + 1) * HB])
                    xh[t][h] = xi
            b2_sb = io.tile([OS, 1], f32)
            nc.sync.dma_start(out=b2_sb, in_=b2s[:, :])

            # weight prep (DVE): ap = sum_k max(w1,0)*w2, amn = -sum_k min(w1,0)*w2
            ap_t, amn_t = [], []
            for t in range(NT):
                mpt = work.tile([128, K, OS], f32, tag=f"mp{t}")
                nc.vector.scalar_tensor_tensor(mpt, w1_t[t], 0.0, w2_t[t],
                                               op0=OP.max, op1=OP.mult)
                mmt = work.tile([128, K, OS], f32, tag=f"mm{t}")
                nc.vector.scalar_tensor_tensor(mmt, w1_t[t], 0.0, w2_t[t],
                                               op0=OP.min, op1=OP.mult)
                ap2 = work.tile([128, 2, OS], f32, tag=f"ap2{t}")
                nc.vector.tensor_add(ap2, mpt[:, 0:2, :], mpt[:, 2:4, :])
                ap = work.tile([128, OS], f32, tag=f"ap{t}")
                nc.vector.tensor_add(ap, ap2[:, 0, :], ap2[:, 1, :])
                am2 = work.tile([128, 2, OS], f32, tag=f"am2{t}")
                nc.vector.tensor_add(am2, mmt[:, 0:2, :], mmt[:, 2:4, :])
                amn = work.tile([128, OS], f32, tag=f"amn{t}")
                nc.vector.scalar_tensor_tensor(amn, am2[:, 0, :], -1.0,
                                               am2[:, 1, :],
                                               op0=OP.mult, op1=OP.subtract)
                ap_t.append(ap)
                amn_t.append(amn)

            # per-half: relu split (ACT) -> 4 matmuls -> epilogue -> store,
            # so half 0's tail hides under half 1's matmuls
            for h in range(2):
                psum = pp.tile([128, HB], f32, tag=f"ps{h}")
                mm = 0
                for t in range(NT):
                    xp = work.tile([128, HB], f32, tag=f"xp{t}{h}")
                    nc.scalar.activation(xp, xh[t][h], AF.Relu)
                    xn = work.tile([128, HB], f32, tag=f"xn{t}{h}")
                    nc.scalar.activation(xn, xh[t][h], AF.Relu, scale=-1.0)
                    nc.tensor.matmul(psum, lhsT=ap_t[t], rhs=xp,
                                     start=(mm == 0), stop=False)
                    mm += 1
                    nc.tensor.matmul(psum, lhsT=amn_t[t], rhs=xn,
                                     start=False, stop=(mm == 2 * NT - 1))
                    mm += 1
                out_sb = work.tile([128, HB], f32, tag=f"out{h}")
                nc.scalar.activation(out_sb, psum, AF.Identity,
                                     bias=b2_sb, scale=float(SCALE))
                nc.sync.dma_start(out=outt[:, h * HB:(h + 1) * HB], in_=out_sb)

    nc.compile()
    _CACHE["nc"] = nc
    return nc


def _kan_numpy(x, w1, b1, w2, b2):
    # exact fallback, chunked over batch to bound memory
    out = np.empty((x.shape[0], w1.shape[0]), dtype=np.float32)
    d = (w1.shape[0] + w1.shape[1]) / 2
    s = 1.0 / np.sqrt(d * w1.shape[2])
    for lo in range(0, x.shape[0], 128):
        hi = min(lo + 128, x.shape[0])
        h = x[lo:hi, None, :, None] * w1[None] + b1[None]
        np.maximum(h, 0.0, out=h)
        out[lo:hi] = np.einsum("boik,oik->bo", h, w2) * s
    return out + b2[None, :]


def kernel(x, w1, b1, w2, b2):
    x = np.ascontiguousarray(x, dtype=np.float32)
    w1 = np.asarray(w1, dtype=np.float32)
    b1 = np.asarray(b1, dtype=np.float32)
    w2 = np.asarray(w2, dtype=np.float32)
    b2 = np.asarray(b2, dtype=np.float32)

    if x.shape != (B, DIN) or w1.shape != (DOUT, DIN, K) or np.any(b1):
        return _kan_numpy(x, w1, b1, w2, b2)

    from concourse.bass_utils import run_bass_kernel_spmd

    nc = _build_nc()

    xT = np.ascontiguousarray(x.T)                      # (DIN, B)
    w1T = np.ascontiguousarray(w1.transpose(1, 2, 0))   # (DIN, K, DOUT)
    w2T = np.ascontiguousarray(w2.transpose(1, 2, 0))

    in_maps = []
    for core in range(N_CORES):
        bi, oj = divmod(core, OG)
        in_maps.append({
            "xt": np.ascontiguousarray(xT[:, bi * BS:(bi + 1) * BS]),
            "w1t": np.ascontiguousarray(w1T[:, :, oj * OS:(oj + 1) * OS]),
            "w2t": np.ascontiguousarray(w2T[:, :, oj * OS:(oj + 1) * OS]),
            "b2s": np.ascontiguousarray(b2[oj * OS:(oj + 1) * OS]).reshape(OS, 1),
        })

    res = run_bass_kernel_spmd(nc, in_maps, core_ids=list(range(N_CORES)))

    out = np.empty((B, DOUT), dtype=np.float32)
    for core in range(N_CORES):
        bi, oj = divmod(core, OG)
        out[bi * BS:(bi + 1) * BS, oj * OS:(oj + 1) * OS] = res.results[core]["outt"].T
    return out



# revision 3
# speedup vs baseline: 1.3974x; 1.3974x over previous
"""KAN layer kernel for TRN2, 8-core SPMD.

Math: out[b,o] = sum_{i,k} relu(x[b,i]*w1[o,i,k] + b1[o,i,k]) * w2[o,i,k] / 32 + b2[o]
With b1 == 0 (guaranteed by the generator) the relu factorizes:
    relu(x*w) = relu(x)*|w| + x*min(w,0)
so the layer collapses to two matmuls with preprocessed weights:
    D[o,i] = sum_k |w1|*w2        M[o,i] = sum_k min(w1,0)*w2
    out = (relu(x) @ D^T + x @ M^T) / 32 + b2

Sharding: 4 batch groups x 2 dout groups (core = bi*2 + oj).
Wire format is bf16 (host casts, device computes bf16 matmuls with f32
psum accumulation); output returns over the wire in bf16 and is upcast
on host. All arithmetic (weight prep, relu, matmul, epilogue) on device.
"""

import numpy as np

B, DIN, DOUT, K = 2048, 256, 256, 4
N_CORES = 8
BG, OG = 4, 2                      # batch groups x dout groups
BS, OS = B // BG, DOUT // OG       # 512 batch rows, 128 dout cols per core
SCALE = 1.0 / np.sqrt(((DOUT + DIN) / 2) * K)   # 1/32
NT = DIN // 128                    # i-tiles (2)
SLAB = K * OS * 2                  # 1024 bf16 cols per i-tile (w1|w2)
WCOLS = 2 + NT * SLAB              # 2 b2-bitcast cols + slabs
NCH = 2                            # batch chunks
CB = BS // NCH                     # 256 batch cols per chunk

_CACHE = {}


def _build_nc():
    if "nc" in _CACHE:
        return _CACHE["nc"]
    import concourse.bacc as bacc
    import concourse.tile as tile
    from concourse import mybir

    f32 = mybir.dt.float32
    bf16 = mybir.dt.bfloat16
    AF = mybir.ActivationFunctionType
    OP = mybir.AluOpType

    nc = bacc.Bacc("TRN2", target_bir_lowering=False, debug=False,
                   num_devices=N_CORES)
    wb = nc.dram_tensor("wb", [128, WCOLS], bf16, kind="ExternalInput")
    xb = nc.dram_tensor("xb", [128, NT, BS], bf16, kind="ExternalInput")
    outb = nc.dram_tensor("outb", [128, BS], bf16, kind="ExternalOutput")

    KO = K * OS                    # 512 cols of one weight tensor slab

    with tile.TileContext(nc) as tc:
        with (
            tc.tile_pool(name="io", bufs=1) as io,
            tc.tile_pool(name="work", bufs=1) as work,
            tc.tile_pool(name="pp", bufs=1, space="PSUM") as pp,
        ):
            wsb = io.tile([128, WCOLS], bf16, tag="wsb")
            xsb = io.tile([128, NT, BS], bf16, tag="xsb")

            # input DMAs: w tile0 (+b2 bits), w tile1, then x chunks
            nc.sync.dma_start(out=wsb[:, 0:2 + SLAB], in_=wb[:, 0:2 + SLAB])
            nc.sync.dma_start(out=wsb[:, 2 + SLAB:WCOLS],
                              in_=wb[:, 2 + SLAB:WCOLS])
            for c in range(NCH):
                nc.sync.dma_start(out=xsb[:, :, c * CB:(c + 1) * CB],
                                  in_=xb[:, :, c * CB:(c + 1) * CB])

            b2ap = wsb[:, 0:2].bitcast(f32)

            # weight prep (DVE): M_t = sum_k min(w1,0)*w2 first (unblocks the
            # raw-x matmuls), then D_t = sum_k |w1|*w2.
            m_t, d_t = [], []
            for t in range(NT):
                w1t = wsb[:, 2 + t * SLAB:2 + t * SLAB + KO]
                w2t = wsb[:, 2 + t * SLAB + KO:2 + (t + 1) * SLAB]
                bk = work.tile([128, KO], bf16, tag=f"bk{t}")
                nc.vector.scalar_tensor_tensor(bk, w1t, 0.0, w2t,
                                               op0=OP.min, op1=OP.mult)
                b2_ = work.tile([128, KO // 2], bf16, tag=f"b2_{t}")
                nc.vector.tensor_add(b2_, bk[:, 0:KO // 2], bk[:, KO // 2:KO])
                mt = work.tile([128, OS], bf16, tag=f"mt{t}")
                nc.vector.tensor_add(mt, b2_[:, 0:OS], b2_[:, OS:2 * OS])
                m_t.append(mt)
                ak = work.tile([128, KO], bf16, tag=f"ak{t}")
                nc.vector.scalar_tensor_tensor(ak, w1t, 0.0, w2t,
                                               op0=OP.max, op1=OP.mult)
                a2_ = work.tile([128, KO // 2], bf16, tag=f"a2_{t}")
                nc.vector.tensor_add(a2_, ak[:, 0:KO // 2], ak[:, KO // 2:KO])
                pt = work.tile([128, OS], bf16, tag=f"pt{t}")
                nc.vector.tensor_add(pt, a2_[:, 0:OS], a2_[:, OS:2 * OS])
                dt = work.tile([128, OS], bf16, tag=f"dt{t}")
                nc.vector.tensor_sub(dt, pt, mt)
                d_t.append(dt)

            # relu(x) for all chunks up front (ACT), one op per chunk
            xpsb = work.tile([128, NT, BS], bf16, tag="xpsb")
            for c in range(NCH):
                nc.scalar.activation(xpsb[:, :, c * CB:(c + 1) * CB],
                                     xsb[:, :, c * CB:(c + 1) * CB], AF.Relu)

            # per chunk: 4 accumulating matmuls -> epilogue -> store
            for c in range(NCH):
                psum = pp.tile([128, CB], f32, tag=f"ps{c}")
                sl = slice(c * CB, (c + 1) * CB)
                nc.tensor.matmul(psum, lhsT=m_t[0], rhs=xsb[:, 0, sl],
                                 start=True, stop=False)
                nc.tensor.matmul(psum, lhsT=m_t[1], rhs=xsb[:, 1, sl],
                                 start=False, stop=False)
                nc.tensor.matmul(psum, lhsT=d_t[0], rhs=xpsb[:, 0, sl],
                                 start=False, stop=False)
                nc.tensor.matmul(psum, lhsT=d_t[1], rhs=xpsb[:, 1, sl],
                                 start=False, stop=True)
                osb = work.tile([128, CB], bf16, tag=f"osb{c}")
                nc.scalar.activation(osb, psum, AF.Identity,
                                     bias=b2ap, scale=float(SCALE))
                nc.sync.dma_start(out=outb[:, sl], in_=osb)

    nc.compile()
    _CACHE["nc"] = nc
    return nc


def _kan_numpy(x, w1, b1, w2, b2):
    # exact fallback, chunked over batch to bound memory
    out = np.empty((x.shape[0], w1.shape[0]), dtype=np.float32)
    d = (w1.shape[0] + w1.shape[1]) / 2
    s = 1.0 / np.sqrt(d * w1.shape[2])
    for lo in range(0, x.shape[0], 128):
        hi = min(lo + 128, x.shape[0])
        h = x[lo:hi, None, :, None] * w1[None] + b1[None]
        np.maximum(h, 0.0, out=h)
        out[lo:hi] = np.einsum("boik,oik->bo", h, w2) * s
    return out + b2[None, :]


def kernel(x, w1, b1, w2, b2):
    x = np.ascontiguousarray(x, dtype=np.float32)
    w1 = np.asarray(w1, dtype=np.float32)
    b1 = np.asarray(b1, dtype=np.float32)
    w2 = np.asarray(w2, dtype=np.float32)
    b2 = np.asarray(b2, dtype=np.float32)

    if x.shape != (B, DIN) or w1.shape != (DOUT, DIN, K) or np.any(b1):
        return _kan_numpy(x, w1, b1, w2, b2)

    from concourse.bass_utils import run_bass_kernel_spmd
    from ml_dtypes import bfloat16

    nc = _build_nc()

    # bf16 wire format, packed per-core blobs
    xT = np.ascontiguousarray(x.T).astype(bfloat16)     # (DIN, B)
    w1T = w1.transpose(1, 2, 0).astype(bfloat16)        # (DIN, K, DOUT)
    w2T = w2.transpose(1, 2, 0).astype(bfloat16)

    in_maps = []
    for core in range(N_CORES):
        bi, oj = divmod(core, OG)
        # w blob: [128, 2 + NT*SLAB]; slab t = [w1_t (k*o flat) | w2_t]
        wcore = np.empty((128, WCOLS), dtype=bfloat16)
        b2f = np.ascontiguousarray(
            b2[oj * OS:(oj + 1) * OS]).reshape(OS, 1)
        wcore[:, 0:2] = b2f.view(np.uint16).view(bfloat16)
        for t in range(NT):
            r = slice(t * 128, (t + 1) * 128)
            o = slice(oj * OS, (oj + 1) * OS)
            wcore[:, 2 + t * SLAB:2 + t * SLAB + K * OS] = \
                w1T[r, :, o].reshape(128, K * OS)
            wcore[:, 2 + t * SLAB + K * OS:2 + (t + 1) * SLAB] = \
                w2T[r, :, o].reshape(128, K * OS)
        # x blob: [128, NT, BS]
        xcore = np.ascontiguousarray(
            xT[:, bi * BS:(bi + 1) * BS].reshape(NT, 128, BS)
            .transpose(1, 0, 2))
        in_maps.append({"wb": wcore, "xb": xcore})

    res = run_bass_kernel_spmd(nc, in_maps, core_ids=list(range(N_CORES)))

    out = np.empty((B, DOUT), dtype=np.float32)
    for core in range(N_CORES):
        bi, oj = divmod(core, OG)
        out[bi * BS:(bi + 1) * BS, oj * OS:(oj + 1) * OS] = \
            res.results[core]["outb"].T.astype(np.float32)
    return out


# revision 5
# speedup vs baseline: 1.5820x; 1.1321x over previous
"""KAN layer kernel for TRN2, 8-core SPMD.

Math: out[b,o] = sum_{i,k} relu(x[b,i]*w1[o,i,k] + b1[o,i,k]) * w2[o,i,k] / 32 + b2[o]
With b1 == 0 (guaranteed by the generator) the relu factorizes via
relu(z) = (z + |z|)/2 and |x*w1| = |x|*|w1|:
    S[i,o] = sum_k w1*w2        T[i,o] = sum_k |w1|*w2
    out = (x @ S + |x| @ T) * (1/64) + b2
so the layer is two bf16 matmuls plus cheap elementwise prep spread
across DVE (products/k-sums), ACT (|.|, epilogue), and GpSimd (one
product chain leg).

Sharding: 4 batch groups x 2 dout groups (core = bi*2 + oj).
Wire format is bf16 (host casts; all arithmetic on device; f32 psum
accumulation); output returns in bf16 and is upcast on host.
"""

import numpy as np

B, DIN, DOUT, K = 2048, 256, 256, 4
N_CORES = 8
BG, OG = 4, 2                      # batch groups x dout groups
BS, OS = B // BG, DOUT // OG       # 512 batch rows, 128 dout cols per core
SCALE = 1.0 / np.sqrt(((DOUT + DIN) / 2) * K)   # 1/32
NT = DIN // 128                    # i-tiles (2)
KO = K * OS                        # 512 cols per weight tensor slab
WCOLS = 2 + NT * 2 * KO            # b2 bits + w1/w2 slabs
NCH = 2                            # batch chunks
CB = BS // NCH                     # 256 batch cols per chunk

_CACHE = {}


def _build_nc():
    if "nc" in _CACHE:
        return _CACHE["nc"]
    import concourse.bacc as bacc
    import concourse.tile as tile
    from concourse import mybir

    f32 = mybir.dt.float32
    bf16 = mybir.dt.bfloat16
    AF = mybir.ActivationFunctionType
    OP = mybir.AluOpType

    nc = bacc.Bacc("TRN2", target_bir_lowering=False, debug=False,
                   num_devices=N_CORES)
    wb = nc.dram_tensor("wb", [128, WCOLS], bf16, kind="ExternalInput")
    xb = nc.dram_tensor("xb", [128, NT, BS], bf16, kind="ExternalInput")
    outb = nc.dram_tensor("outb", [128, BS], bf16, kind="ExternalOutput")

    def w1c(t):
        return slice(2 + t * 2 * KO, 2 + t * 2 * KO + KO)

    def w2c(t):
        return slice(2 + t * 2 * KO + KO, 2 + (t + 1) * 2 * KO)

    with tile.TileContext(nc) as tc:
        with (
            tc.tile_pool(name="io", bufs=1) as io,
            tc.tile_pool(name="work", bufs=1) as work,
            tc.tile_pool(name="pp", bufs=1, space="PSUM") as pp,
        ):
            wsb = io.tile([128, WCOLS], bf16, tag="wsb")
            xsb = io.tile([128, NT, BS], bf16, tag="xsb")

            # inputs: [b2+w1t0+w2t0] and [w1t1+w2t1] from SP, x halves from ACT
            nc.sync.dma_start(out=wsb[:, 0:2 + 2 * KO], in_=wb[:, 0:2 + 2 * KO])
            nc.sync.dma_start(out=wsb[:, 2 + 2 * KO:WCOLS],
                              in_=wb[:, 2 + 2 * KO:WCOLS])
            for c in range(NCH):
                nc.scalar.dma_start(out=xsb[:, :, c * CB:(c + 1) * CB],
                                    in_=xb[:, :, c * CB:(c + 1) * CB])

            b2ap = wsb[:, 0:2].bitcast(f32)

            # |w1| per tile on ACT
            a4 = []
            for t in range(NT):
                a = work.tile([128, KO], bf16, tag=f"a4{t}")
                nc.scalar.activation(a, wsb[:, w1c(t)], AF.Abs)
                a4.append(a)

            # S chains on DVE
            s_t, t_t = [], []
            for t in range(NT):
                s4 = work.tile([128, KO], bf16, tag=f"s4{t}")
                nc.vector.tensor_mul(s4, wsb[:, w1c(t)], wsb[:, w2c(t)])
                s2 = work.tile([128, KO // 2], bf16, tag=f"s2{t}")
                nc.vector.tensor_add(s2, s4[:, 0:KO // 2], s4[:, KO // 2:KO])
                st = work.tile([128, OS], bf16, tag=f"st{t}")
                nc.vector.tensor_add(st, s2[:, 0:OS], s2[:, OS:2 * OS])
                s_t.append(st)

            # T tile0 product on GpSimd (parallel with DVE), adds on DVE
            t4_0 = work.tile([128, KO], bf16, tag="t4_0")
            nc.gpsimd.tensor_mul(t4_0, a4[0], wsb[:, w2c(0)])
            t2_0 = work.tile([128, KO // 2], bf16, tag="t2_0")
            nc.vector.tensor_add(t2_0, t4_0[:, 0:KO // 2], t4_0[:, KO // 2:KO])
            tt_0 = work.tile([128, OS], bf16, tag="tt_0")
            nc.vector.tensor_add(tt_0, t2_0[:, 0:OS], t2_0[:, OS:2 * OS])
            t_t.append(tt_0)
            # T tile1 chain fully on DVE
            t4_1 = work.tile([128, KO], bf16, tag="t4_1")
            nc.vector.tensor_mul(t4_1, a4[1], wsb[:, w2c(1)])
            t2_1 = work.tile([128, KO // 2], bf16, tag="t2_1")
            nc.vector.tensor_add(t2_1, t4_1[:, 0:KO // 2], t4_1[:, KO // 2:KO])
            tt_1 = work.tile([128, OS], bf16, tag="tt_1")
            nc.vector.tensor_add(tt_1, t2_1[:, 0:OS], t2_1[:, OS:2 * OS])
            t_t.append(tt_1)

            # |x| for both chunks in one ACT op (after x fully arrives)
            xa = work.tile([128, NT, BS], bf16, tag="xa")
            nc.scalar.activation(xa, xsb, AF.Abs)

            # per chunk: 4 accumulating matmuls, S parts first, T1 last
            psums = []
            for c in range(NCH):
                psum = pp.tile([128, CB], f32, tag=f"ps{c}")
                sl = slice(c * CB, (c + 1) * CB)
                nc.tensor.matmul(psum, lhsT=s_t[0], rhs=xsb[:, 0, sl],
                                 start=True, stop=False)
                nc.tensor.matmul(psum, lhsT=s_t[1], rhs=xsb[:, 1, sl],
                                 start=False, stop=False)
                nc.tensor.matmul(psum, lhsT=t_t[0], rhs=xa[:, 0, sl],
                                 start=False, stop=False)
                nc.tensor.matmul(psum, lhsT=t_t[1], rhs=xa[:, 1, sl],
                                 start=False, stop=True)
                psums.append(psum)

            s2c = float(SCALE) / 2.0
            osb = work.tile([128, BS], bf16, tag="osb")
            # chunk 0 epilogue on ACT, chunk 1 on DVE (parallel)
            nc.scalar.activation(osb[:, 0:CB], psums[0], AF.Identity,
                                 bias=b2ap, scale=s2c)
            nc.vector.tensor_scalar(osb[:, CB:BS], psums[1], s2c, b2ap,
                                    op0=OP.mult, op1=OP.add)
            nc.sync.dma_start(out=outb[:, :], in_=osb)

    nc.compile()
    _CACHE["nc"] = nc
    return nc


def _kan_numpy(x, w1, b1, w2, b2):
    # exact fallback, chunked over batch to bound memory
    out = np.empty((x.shape[0], w1.shape[0]), dtype=np.float32)
    d = (w1.shape[0] + w1.shape[1]) / 2
    s = 1.0 / np.sqrt(d * w1.shape[2])
    for lo in range(0, x.shape[0], 128):
        hi = min(lo + 128, x.shape[0])
        h = x[lo:hi, None, :, None] * w1[None] + b1[None]
        np.maximum(h, 0.0, out=h)
        out[lo:hi] = np.einsum("boik,oik->bo", h, w2) * s
    return out + b2[None, :]


def kernel(x, w1, b1, w2, b2):
    x = np.ascontiguousarray(x, dtype=np.float32)
    w1 = np.asarray(w1, dtype=np.float32)
    b1 = np.asarray(b1, dtype=np.float32)
    w2 = np.asarray(w2, dtype=np.float32)
    b2 = np.asarray(b2, dtype=np.float32)

    if x.shape != (B, DIN) or w1.shape != (DOUT, DIN, K) or np.any(b1):
        return _kan_numpy(x, w1, b1, w2, b2)

    from concourse.bass_utils import run_bass_kernel_spmd
    from ml_dtypes import bfloat16

    nc = _build_nc()

    xT = np.ascontiguousarray(x.T).astype(bfloat16)     # (DIN, B)
    w1T = w1.transpose(1, 2, 0).astype(bfloat16)        # (DIN, K, DOUT)
    w2T = w2.transpose(1, 2, 0).astype(bfloat16)

    in_maps = []
    for core in range(N_CORES):
        bi, oj = divmod(core, OG)
        wcore = np.empty((128, WCOLS), dtype=bfloat16)
        b2f = np.ascontiguousarray(
            b2[oj * OS:(oj + 1) * OS]).reshape(OS, 1)
        wcore[:, 0:2] = b2f.view(np.uint16).view(bfloat16)
        for t in range(NT):
            r = slice(t * 128, (t + 1) * 128)
            o = slice(oj * OS, (oj + 1) * OS)
            wcore[:, 2 + t * 2 * KO:2 + t * 2 * KO + KO] = \
                w1T[r, :, o].reshape(128, KO)
            wcore[:, 2 + t * 2 * KO + KO:2 + (t + 1) * 2 * KO] = \
                w2T[r, :, o].reshape(128, KO)
        xcore = np.ascontiguousarray(
            xT[:, bi * BS:(bi + 1) * BS].reshape(NT, 128, BS)
            .transpose(1, 0, 2))
        in_maps.append({"wb": wcore, "xb": xcore})

    res = run_bass_kernel_spmd(nc, in_maps, core_ids=list(range(N_CORES)))

    out = np.empty((B, DOUT), dtype=np.float32)
    for core in range(N_CORES):
        bi, oj = divmod(core, OG)
        out[bi * BS:(bi + 1) * BS, oj * OS:(oj + 1) * OS] = \
            res.results[core]["outb"].T.astype(np.float32)
    return out


# revision 9
# speedup vs baseline: 1.7551x; 1.1094x over previous
"""KAN layer kernel for TRN2, 8-core SPMD.

Math: out[b,o] = sum_{i,k} relu(x[b,i]*w1[o,i,k] + b1[o,i,k]) * w2[o,i,k] / 32 + b2[o]
With b1 == 0 (guaranteed by the generator) the relu factorizes via
relu(z) = (z + |z|)/2 and |x*w1| = |x|*|w1|:
    S[i,o] = sum_k w1*w2        T[i,o] = sum_k |w1|*w2
    out = (x @ S + |x| @ T) * (1/64) + b2
Two bf16 matmuls plus elementwise prep spread across DVE (products,
k-sums), ACT (|.|, epilogue), GpSimd (one product leg). The output store
is a prepared SWDGE scatter (identity indices) fired by trigger_dma,
which skips the HWDGE + DGE-delay latency of a normal DMA; the scatter
ADDs into the zero-initialized output buffer, which equals a store.

Sharding: 4 batch groups x 2 dout groups (core = bi*2 + oj).
Wire format is bf16 (host casts; all arithmetic on device; f32 psum
accumulation); output returns in bf16 and is upcast on host.
"""

import numpy as np

B, DIN, DOUT, K = 2048, 256, 256, 4
N_CORES = 8
BG, OG = 4, 2                      # batch groups x dout groups
BS, OS = B // BG, DOUT // OG       # 512 batch rows, 128 dout cols per core
SCALE = 1.0 / np.sqrt(((DOUT + DIN) / 2) * K)   # 1/32
NT = DIN // 128                    # i-tiles (2)
KO = K * OS                        # 512 cols per weight tensor slab
NIDX = 8                           # identity scatter idxs: 8 int16 cols
WCOLS = 2 + NT * 2 * KO + NIDX     # b2 bits + w1/w2 slabs + idxs
NCH = 2                            # batch chunks
CB = BS // NCH                     # 256 batch cols per chunk

_CACHE = {}


def _build_nc():
    if "nc" in _CACHE:
        return _CACHE["nc"]
    import concourse.bacc as bacc
    import concourse.tile as tile
    from concourse import mybir

    f32 = mybir.dt.float32
    bf16 = mybir.dt.bfloat16
    i16 = mybir.dt.int16
    AF = mybir.ActivationFunctionType
    OP = mybir.AluOpType

    nc = bacc.Bacc("TRN2", target_bir_lowering=False, debug=False,
                   num_devices=N_CORES)
    wb = nc.dram_tensor("wb", [128, WCOLS], bf16, kind="ExternalInput")
    xb = nc.dram_tensor("xb", [128, NT, BS], bf16, kind="ExternalInput")
    outb = nc.dram_tensor("outb", [128, BS], bf16, kind="ExternalOutput")

    W0E = 2 + 2 * KO               # end of [b2 | w1t0 | w2t0]

    def w1c(t):
        return slice(2 + t * 2 * KO, 2 + t * 2 * KO + KO)

    def w2c(t):
        return slice(2 + t * 2 * KO + KO, 2 + (t + 1) * 2 * KO)

    dma_sem = nc.alloc_semaphore("out_dma_sem")

    with tile.TileContext(nc) as tc:
        with (
            tc.tile_pool(name="io", bufs=1) as io,
            tc.tile_pool(name="work", bufs=1) as work,
            tc.tile_pool(name="pp", bufs=1, space="PSUM") as pp,
        ):
            wsb = io.tile([128, WCOLS], bf16, tag="wsb")
            xsb = io.tile([128, NT, BS], bf16, tag="xsb")

            # inputs: [b2|w1t0|w2t0] from SP; [w1t1|w2t1|idxs] and x halves
            # from ACT so HWDGE order is wt0, wt1, x0, x1
            nc.sync.dma_start(out=wsb[:, 0:W0E], in_=wb[:, 0:W0E])
            nc.scalar.dma_start(out=wsb[:, W0E:WCOLS], in_=wb[:, W0E:WCOLS])
            for c in range(NCH):
                nc.scalar.dma_start(out=xsb[:, :, c * CB:(c + 1) * CB],
                                    in_=xb[:, :, c * CB:(c + 1) * CB])

            b2ap = wsb[:, 0:2].bitcast(f32)
            idxs_ap = wsb[:, WCOLS - NIDX:WCOLS].bitcast(i16)

            # |w1| per tile on ACT
            a4 = []
            for t in range(NT):
                a = work.tile([128, KO], bf16, tag=f"a4{t}")
                nc.scalar.activation(a, wsb[:, w1c(t)], AF.Abs)
                a4.append(a)

            # S chains on DVE
            s_t, t_t = [], []
            for t in range(NT):
                s4 = work.tile([128, KO], bf16, tag=f"s4{t}")
                nc.vector.tensor_mul(s4, wsb[:, w1c(t)], wsb[:, w2c(t)])
                s2 = work.tile([128, KO // 2], bf16, tag=f"s2{t}")
                nc.vector.tensor_add(s2, s4[:, 0:KO // 2], s4[:, KO // 2:KO])
                st = work.tile([128, OS], bf16, tag=f"st{t}")
                nc.vector.tensor_add(st, s2[:, 0:OS], s2[:, OS:2 * OS])
                s_t.append(st)

            # T tile0 product on GpSimd (parallel with DVE's chains)
            t4_0 = work.tile([128, KO], bf16, tag="t4_0")
            nc.gpsimd.tensor_mul(t4_0, a4[0], wsb[:, w2c(0)])
            # T tile1 product + both tiles' adds on DVE
            t4_1 = work.tile([128, KO], bf16, tag="t4_1")
            nc.vector.tensor_mul(t4_1, a4[1], wsb[:, w2c(1)])
            t2_0 = work.tile([128, KO // 2], bf16, tag="t2_0")
            nc.vector.tensor_add(t2_0, t4_0[:, 0:KO // 2], t4_0[:, KO // 2:KO])
            tt_0 = work.tile([128, OS], bf16, tag="tt_0")
            nc.vector.tensor_add(tt_0, t2_0[:, 0:OS], t2_0[:, OS:2 * OS])
            t_t.append(tt_0)
            t2_1 = work.tile([128, KO // 2], bf16, tag="t2_1")
            nc.vector.tensor_add(t2_1, t4_1[:, 0:KO // 2], t4_1[:, KO // 2:KO])
            tt_1 = work.tile([128, OS], bf16, tag="tt_1")
            nc.vector.tensor_add(tt_1, t2_1[:, 0:OS], t2_1[:, OS:2 * OS])
            t_t.append(tt_1)

            # |x| for both chunks in one ACT op
            xa = work.tile([128, NT, BS], bf16, tag="xa")
            nc.scalar.activation(xa, xsb, AF.Abs)

            # matmuls: S parts of both chunks first, T parts after
            psums = []
            for c in range(NCH):
                psum = pp.tile([128, CB], f32, tag=f"ps{c}")
                psums.append(psum)
            for c in range(NCH):
                sl = slice(c * CB, (c + 1) * CB)
                nc.tensor.matmul(psums[c], lhsT=s_t[0], rhs=xsb[:, 0, sl],
                                 start=True, stop=False)
                nc.tensor.matmul(psums[c], lhsT=s_t[1], rhs=xsb[:, 1, sl],
                                 start=False, stop=False)
            for c in range(NCH):
                sl = slice(c * CB, (c + 1) * CB)
                nc.tensor.matmul(psums[c], lhsT=t_t[0], rhs=xa[:, 0, sl],
                                 start=False, stop=False)
                nc.tensor.matmul(psums[c], lhsT=t_t[1], rhs=xa[:, 1, sl],
                                 start=False, stop=True)

            s2c = float(SCALE) / 2.0
            osb = work.tile([128, 1, BS], bf16, tag="osb")
            # chunk 0 epilogue on ACT, chunk 1 on DVE (parallel)
            nc.scalar.activation(osb[:, 0, 0:CB], psums[0], AF.Identity,
                                 bias=b2ap, scale=s2c)
            nc.vector.tensor_scalar(osb[:, 0, CB:BS], psums[1], s2c, b2ap,
                                    op0=OP.mult, op1=OP.add)

            # output: prepared SWDGE scatter (identity idxs) + trigger
            nc.gpsimd.dma_scatter_add(outb[:, :], osb, idxs_ap, 128, 128, BS,
                                      prepare_only=True, sem=dma_sem)
            nc.gpsimd.trigger_dma(count=None)

    nc.compile()

    # Tile assigns the prepared scatter a DMASW lane and makes the block
    # exit wait on that lane semaphore, but routes the user sem= into
    # on_update[0] instead of the lane sem — nothing ever bumps the lane.
    # Point the prep's DMA-completion update at the lane semaphore (the
    # same attachment a normal Pool DMA gets), matching the exit wait.
    fn = nc.m.functions[0]
    prep = None
    lane_id = lane_name = None
    for blk in fn.blocks:
        for inst in blk.instructions:
            if type(inst).__name__ == "InstDMAScatterAddAnt":
                prep = inst
            si = inst.sync_info
            if si is not None:
                for w in (si.on_wait or []):
                    nm = getattr(w, "ant_name", None)
                    if nm and nm.startswith("DMASW") and w.wait_value == 16:
                        lane_id, lane_name = w.id, nm
    assert prep is not None and lane_id is not None
    u0 = prep.sync_info.on_update[0]
    u0.id = lane_id
    u0.ant_name = lane_name

    _CACHE["nc"] = nc
    return nc


def _kan_numpy(x, w1, b1, w2, b2):
    # exact fallback, chunked over batch to bound memory
    out = np.empty((x.shape[0], w1.shape[0]), dtype=np.float32)
    d = (w1.shape[0] + w1.shape[1]) / 2
    s = 1.0 / np.sqrt(d * w1.shape[2])
    for lo in range(0, x.shape[0], 128):
        hi = min(lo + 128, x.shape[0])
        h = x[lo:hi, None, :, None] * w1[None] + b1[None]
        np.maximum(h, 0.0, out=h)
        out[lo:hi] = np.einsum("boik,oik->bo", h, w2) * s
    return out + b2[None, :]


def kernel(x, w1, b1, w2, b2):
    x = np.ascontiguousarray(x, dtype=np.float32)
    w1 = np.asarray(w1, dtype=np.float32)
    b1 = np.asarray(b1, dtype=np.float32)
    w2 = np.asarray(w2, dtype=np.float32)
    b2 = np.asarray(b2, dtype=np.float32)

    if x.shape != (B, DIN) or w1.shape != (DOUT, DIN, K) or np.any(b1):
        return _kan_numpy(x, w1, b1, w2, b2)

    from concourse.bass_utils import run_bass_kernel_spmd
    from ml_dtypes import bfloat16

    nc = _build_nc()

    xT = np.ascontiguousarray(x.T).astype(bfloat16)     # (DIN, B)
    w1T = w1.transpose(1, 2, 0).astype(bfloat16)        # (DIN, K, DOUT)
    w2T = w2.transpose(1, 2, 0).astype(bfloat16)
    # token i reads its destination from idxs[i % 16, i // 16]
    idxs = np.tile(
        np.ascontiguousarray(np.arange(128, dtype=np.int16).reshape(NIDX, 16).T),
        (8, 1))

    in_maps = []
    for core in range(N_CORES):
        bi, oj = divmod(core, OG)
        wcore = np.empty((128, WCOLS), dtype=bfloat16)
        b2f = np.ascontiguousarray(
            b2[oj * OS:(oj + 1) * OS]).reshape(OS, 1)
        wcore[:, 0:2] = b2f.view(np.uint16).view(bfloat16)
        for t in range(NT):
            r = slice(t * 128, (t + 1) * 128)
            o = slice(oj * OS, (oj + 1) * OS)
            wcore[:, 2 + t * 2 * KO:2 + t * 2 * KO + KO] = \
                w1T[r, :, o].reshape(128, KO)
            wcore[:, 2 + t * 2 * KO + KO:2 + (t + 1) * 2 * KO] = \
                w2T[r, :, o].reshape(128, KO)
        wcore[:, WCOLS - NIDX:WCOLS] = idxs.view(bfloat16)
        xcore = np.ascontiguousarray(
            xT[:, bi * BS:(bi + 1) * BS].reshape(NT, 128, BS)
            .transpose(1, 0, 2))
        in_maps.append({"wb": wcore, "xb": xcore})

    res = run_bass_kernel_spmd(nc, in_maps, core_ids=list(range(N_CORES)))

    out = np.empty((B, DOUT), dtype=np.float32)
    for core in range(N_CORES):
        bi, oj = divmod(core, OG)
        out[bi * BS:(bi + 1) * BS, oj * OS:(oj + 1) * OS] = \
            res.results[core]["outb"].T.astype(np.float32)
    return out


# revision 10
# speedup vs baseline: 1.7748x; 1.0112x over previous
"""KAN layer kernel for TRN2, 8-core SPMD.

Math: out[b,o] = sum_{i,k} relu(x[b,i]*w1[o,i,k] + b1[o,i,k]) * w2[o,i,k] / 32 + b2[o]
With b1 == 0 (guaranteed by the generator) the relu factorizes via
relu(z) = (z + |z|)/2 and |x*w1| = |x|*|w1|:
    S[i,o] = sum_k w1*w2        T[i,o] = sum_k |w1|*w2
    out = (x @ S + |x| @ T) * (1/64) + b2
Two bf16 matmuls plus elementwise prep spread across DVE (products,
k-sums), ACT (|.|, epilogue), GpSimd (one product leg). The output store
is a prepared SWDGE scatter (identity indices) fired by trigger_dma,
which skips the HWDGE + DGE-delay latency of a normal DMA; the scatter
ADDs into the zero-initialized output buffer, which equals a store.

Sharding: 4 batch groups x 2 dout groups (core = bi*2 + oj).
Wire format is bf16 (host casts; all arithmetic on device; f32 psum
accumulation); output returns in bf16 and is upcast on host.
"""

import numpy as np

B, DIN, DOUT, K = 2048, 256, 256, 4
N_CORES = 8
BG, OG = 4, 2                      # batch groups x dout groups
BS, OS = B // BG, DOUT // OG       # 512 batch rows, 128 dout cols per core
SCALE = 1.0 / np.sqrt(((DOUT + DIN) / 2) * K)   # 1/32
NT = DIN // 128                    # i-tiles (2)
KO = K * OS                        # 512 cols per weight tensor slab
NIDX = 8                           # identity scatter idxs: 8 int16 cols
WCOLS = 2 + NT * 2 * KO + NIDX     # b2 bits + w1/w2 slabs + idxs
NCH = 2                            # batch chunks
CB = BS // NCH                     # 256 batch cols per chunk

_CACHE = {}


def _build_nc():
    if "nc" in _CACHE:
        return _CACHE["nc"]
    import concourse.bacc as bacc
    import concourse.tile as tile
    from concourse import mybir

    f32 = mybir.dt.float32
    bf16 = mybir.dt.bfloat16
    i16 = mybir.dt.int16
    AF = mybir.ActivationFunctionType
    OP = mybir.AluOpType

    nc = bacc.Bacc("TRN2", target_bir_lowering=False, debug=False,
                   num_devices=N_CORES)
    wb = nc.dram_tensor("wb", [128, WCOLS], bf16, kind="ExternalInput")
    xb = nc.dram_tensor("xb", [128, NT, BS], bf16, kind="ExternalInput")
    outb = nc.dram_tensor("outb", [128, BS], bf16, kind="ExternalOutput")

    W0E = 2 + 2 * KO               # end of [b2 | w1t0 | w2t0]

    def w1c(t):
        return slice(2 + t * 2 * KO, 2 + t * 2 * KO + KO)

    def w2c(t):
        return slice(2 + t * 2 * KO + KO, 2 + (t + 1) * 2 * KO)

    dma_sem = nc.alloc_semaphore("out_dma_sem")

    with tile.TileContext(nc) as tc:
        with (
            tc.tile_pool(name="io", bufs=1) as io,
            tc.tile_pool(name="work", bufs=1) as work,
            tc.tile_pool(name="pp", bufs=1, space="PSUM") as pp,
        ):
            wsb = io.tile([128, WCOLS], bf16, tag="wsb")
            xsb = io.tile([128, NT, BS], bf16, tag="xsb")

            # inputs: [b2|w1t0|w2t0] from SP; [w1t1|w2t1|idxs] and x halves
            # from ACT so HWDGE order is wt0, wt1, x0, x1
            nc.sync.dma_start(out=wsb[:, 0:W0E], in_=wb[:, 0:W0E])
            nc.scalar.dma_start(out=wsb[:, W0E:WCOLS], in_=wb[:, W0E:WCOLS])
            for c in range(NCH):
                nc.scalar.dma_start(out=xsb[:, :, c * CB:(c + 1) * CB],
                                    in_=xb[:, :, c * CB:(c + 1) * CB])

            b2ap = wsb[:, 0:2].bitcast(f32)
            idxs_ap = wsb[:, WCOLS - NIDX:WCOLS].bitcast(i16)

            # |w1| per tile on ACT
            a4 = []
            for t in range(NT):
                a = work.tile([128, KO], bf16, tag=f"a4{t}")
                nc.scalar.activation(a, wsb[:, w1c(t)], AF.Abs)
                a4.append(a)

            # S chains on DVE
            s_t, t_t = [], []
            for t in range(NT):
                s4 = work.tile([128, KO], bf16, tag=f"s4{t}")
                nc.vector.tensor_mul(s4, wsb[:, w1c(t)], wsb[:, w2c(t)])
                s2 = work.tile([128, KO // 2], bf16, tag=f"s2{t}")
                nc.vector.tensor_add(s2, s4[:, 0:KO // 2], s4[:, KO // 2:KO])
                st = work.tile([128, OS], bf16, tag=f"st{t}")
                nc.vector.tensor_add(st, s2[:, 0:OS], s2[:, OS:2 * OS])
                s_t.append(st)

            # T tile0 product on GpSimd (parallel with DVE's chains)
            t4_0 = work.tile([128, KO], bf16, tag="t4_0")
            nc.gpsimd.tensor_mul(t4_0, a4[0], wsb[:, w2c(0)])
            # T tile1 product + both tiles' adds on DVE
            t4_1 = work.tile([128, KO], bf16, tag="t4_1")
            nc.vector.tensor_mul(t4_1, a4[1], wsb[:, w2c(1)])
            t2_0 = work.tile([128, KO // 2], bf16, tag="t2_0")
            nc.vector.tensor_add(t2_0, t4_0[:, 0:KO // 2], t4_0[:, KO // 2:KO])
            tt_0 = work.tile([128, OS], bf16, tag="tt_0")
            nc.vector.tensor_add(tt_0, t2_0[:, 0:OS], t2_0[:, OS:2 * OS])
            t_t.append(tt_0)
            t2_1 = work.tile([128, KO // 2], bf16, tag="t2_1")
            nc.vector.tensor_add(t2_1, t4_1[:, 0:KO // 2], t4_1[:, KO // 2:KO])
            tt_1 = work.tile([128, OS], bf16, tag="tt_1")
            nc.vector.tensor_add(tt_1, t2_1[:, 0:OS], t2_1[:, OS:2 * OS])
            t_t.append(tt_1)

            # |x| per chunk on ACT (chunk 0 available before chunk 1 lands)
            xa = work.tile([128, NT, BS], bf16, tag="xa")
            for c in range(NCH):
                nc.scalar.activation(xa[:, :, c * CB:(c + 1) * CB],
                                     xsb[:, :, c * CB:(c + 1) * CB], AF.Abs)

            # matmuls: S parts of both chunks first, T parts after
            psums = []
            for c in range(NCH):
                psum = pp.tile([128, CB], f32, tag=f"ps{c}")
                psums.append(psum)
            for c in range(NCH):
                sl = slice(c * CB, (c + 1) * CB)
                nc.tensor.matmul(psums[c], lhsT=s_t[0], rhs=xsb[:, 0, sl],
                                 start=True, stop=False)
                nc.tensor.matmul(psums[c], lhsT=s_t[1], rhs=xsb[:, 1, sl],
                                 start=False, stop=False)
            for c in range(NCH):
                sl = slice(c * CB, (c + 1) * CB)
                nc.tensor.matmul(psums[c], lhsT=t_t[0], rhs=xa[:, 0, sl],
                                 start=False, stop=False)
                nc.tensor.matmul(psums[c], lhsT=t_t[1], rhs=xa[:, 1, sl],
                                 start=False, stop=True)

            s2c = float(SCALE) / 2.0
            osb = work.tile([128, 1, BS], bf16, tag="osb")
            # chunk 0 epilogue on ACT, chunk 1 on DVE (parallel)
            nc.scalar.activation(osb[:, 0, 0:CB], psums[0], AF.Identity,
                                 bias=b2ap, scale=s2c)
            nc.vector.tensor_scalar(osb[:, 0, CB:BS], psums[1], s2c, b2ap,
                                    op0=OP.mult, op1=OP.add)

            # output: prepared SWDGE scatter (identity idxs) + trigger
            nc.gpsimd.dma_scatter_add(outb[:, :], osb, idxs_ap, 128, 128, BS,
                                      prepare_only=True, sem=dma_sem)
            nc.gpsimd.trigger_dma(count=None)

    nc.compile()

    # Tile assigns the prepared scatter a DMASW lane and makes the block
    # exit wait on that lane semaphore, but routes the user sem= into
    # on_update[0] instead of the lane sem — nothing ever bumps the lane.
    # Point the prep's DMA-completion update at the lane semaphore (the
    # same attachment a normal Pool DMA gets), matching the exit wait.
    fn = nc.m.functions[0]
    prep = None
    lane_id = lane_name = None
    for blk in fn.blocks:
        for inst in blk.instructions:
            if type(inst).__name__ == "InstDMAScatterAddAnt":
                prep = inst
            si = inst.sync_info
            if si is not None:
                for w in (si.on_wait or []):
                    nm = getattr(w, "ant_name", None)
                    if nm and nm.startswith("DMASW") and w.wait_value == 16:
                        lane_id, lane_name = w.id, nm
    assert prep is not None and lane_id is not None
    u0 = prep.sync_info.on_update[0]
    u0.id = lane_id
    u0.ant_name = lane_name

    _CACHE["nc"] = nc
    return nc


def _kan_numpy(x, w1, b1, w2, b2):
    # exact fallback, chunked over batch to bound memory
    out = np.empty((x.shape[0], w1.shape[0]), dtype=np.float32)
    d = (w1.shape[0] + w1.shape[1]) / 2
    s = 1.0 / np.sqrt(d * w1.shape[2])
    for lo in range(0, x.shape[0], 128):
        hi = min(lo + 128, x.shape[0])
        h = x[lo:hi, None, :, None] * w1[None] + b1[None]
        np.maximum(h, 0.0, out=h)
        out[lo:hi] = np.einsum("boik,oik->bo", h, w2) * s
    return out + b2[None, :]


def kernel(x, w1, b1, w2, b2):
    x = np.ascontiguousarray(x, dtype=np.float32)
    w1 = np.asarray(w1, dtype=np.float32)
    b1 = np.asarray(b1, dtype=np.float32)
    w2 = np.asarray(w2, dtype=np.float32)
    b2 = np.asarray(b2, dtype=np.float32)

    if x.shape != (B, DIN) or w1.shape != (DOUT, DIN, K) or np.any(b1):
        return _kan_numpy(x, w1, b1, w2, b2)

    from concourse.bass_utils import run_bass_kernel_spmd
    from ml_dtypes import bfloat16

    nc = _build_nc()

    xT = np.ascontiguousarray(x.T).astype(bfloat16)     # (DIN, B)
    w1T = w1.transpose(1, 2, 0).astype(bfloat16)        # (DIN, K, DOUT)
    w2T = w2.transpose(1, 2, 0).astype(bfloat16)
    # token i reads its destination from idxs[i % 16, i // 16]
    idxs = np.tile(
        np.ascontiguousarray(np.arange(128, dtype=np.int16).reshape(NIDX, 16).T),
        (8, 1))

    in_maps = []
    for core in range(N_CORES):
        bi, oj = divmod(core, OG)
        wcore = np.empty((128, WCOLS), dtype=bfloat16)
        b2f = np.ascontiguousarray(
            b2[oj * OS:(oj + 1) * OS]).reshape(OS, 1)
        wcore[:, 0:2] = b2f.view(np.uint16).view(bfloat16)
        for t in range(NT):
            r = slice(t * 128, (t + 1) * 128)
            o = slice(oj * OS, (oj + 1) * OS)
            wcore[:, 2 + t * 2 * KO:2 + t * 2 * KO + KO] = \
                w1T[r, :, o].reshape(128, KO)
            wcore[:, 2 + t * 2 * KO + KO:2 + (t + 1) * 2 * KO] = \
                w2T[r, :, o].reshape(128, KO)
        wcore[:, WCOLS - NIDX:WCOLS] = idxs.view(bfloat16)
        xcore = np.ascontiguousarray(
            xT[:, bi * BS:(bi + 1) * BS].reshape(NT, 128, BS)
            .transpose(1, 0, 2))
        in_maps.append({"wb": wcore, "xb": xcore})

    res = run_bass_kernel_spmd(nc, in_maps, core_ids=list(range(N_CORES)))

    out = np.empty((B, DOUT), dtype=np.float32)
    for core in range(N_CORES):
        bi, oj = divmod(core, OG)
        out[bi * BS:(bi + 1) * BS, oj * OS:(oj + 1) * OS] = \
            res.results[core]["outb"].T.astype(np.float32)
    return out
